# revision 13
# baseline (speedup 1.0000x reference)
"""Trainium2 Bass kernel for batched dot-product attention with query-row
masking (nn_DotProductAttention: B=32, Q=K=2048, D=128, fp32).

Strategy (v2)
-------------
- 2D sharding over 8 cores: 2 batch-groups x 4 K-quarters. Batches are
  sorted by valid_len and paired into 16 slots (one batch per group per
  slot); each core handles its group's 16 batches against one 512-key
  quarter. Per-core query extent = sum of pairwise slot maxes (~16.8k rows)
  -- ~20% less PE work than 8-way batch-parallel slot maxes, and the
  outputs are partial (num, den) sums the host combines, so the split
  needs no collectives.
- The reference masks whole QUERY rows; mask and 1/sqrt(D) are folded into
  Q on the host (masked rows -> zero queries -> uniform softmax), and rows
  beyond each slot's extent are filled on the host with mean(V).
- Device per chunk (512 q rows): fp16 score matmuls per 128-key tile into
  [128, 2, 512] PSUM tiles; exp is split between ScalarE (native Exp,
  bias -1) and DVE (Schraudolph bit-trick: i16 = trunc(1477.32*s +
  13883.18) bitcast to fp16 ~= exp(s-1)), chosen per tile by a running
  busy-time balance -- this halves the ScalarE exp wall that bounded v1.
- PV: fp16 matmuls with the exp tiles as weights against [V | 1] (the ones
  column accumulates the softmax denominator); one PSUM accumulation group
  spans two 128-row subtiles of one bank ([128, 2, 129]). PSUM->SBUF exit
  copies (fp16) are also balance-assigned to ScalarE/DVE. Normalization
  (num/den) happens on the host, which also sums the 4 K-quarter partials.
- DMA: K/V for all 16 slots are SBUF-resident (loaded once at start); Q
  chunks stream on the SP HWDGE queue; V loads and output stores ride the
  gpsimd SWDGE queues with stores emitted two chunks late (in-order queue
  must not park on unfinished PV results); tail flush on SP.
"""

import sys

for _p in ("/opt/trn_rl_repo", "/root/.axon_site/_ro/trn_rl_repo"):
    if _p not in sys.path:
        sys.path.append(_p)

import math
from contextlib import ExitStack

import numpy as np

import concourse.bacc as bacc
import concourse.tile as tile
from concourse import mybir
from concourse.bass_utils import run_bass_kernel_spmd

B, S, D = 32, 2048, 128
N_CORES = 8
NG = 2                    # batch groups
NSLOT = B // NG           # 16 slots (one batch per group each)
KQ = N_CORES // NG        # 4 K-quarters
KK = S // KQ              # 512 keys per core
NKT = KK // 128           # 4 key tiles per core
F32 = mybir.dt.float32
F16 = mybir.dt.float16
I16 = mybir.dt.int16

A16 = 1024.0 / math.log(2.0)          # 1477.3196
SCH_BIAS = 15360.0 + 0.5              # schraudolph exp(s), trunc-centered

_COMPILED = {}

# rough per-instruction busy-ns models used for ACT/DVE load balancing
def _exp_cost_act(cols):
    return cols * 0.8333 + 420.0

def _exp_cost_dve(cols):
    return cols * 1.0417 + 300.0

def _cp_cost_act(cols):
    return cols * 0.8333 + 380.0

def _cp_cost_dve(cols):
    return cols * 1.0417 + 250.0


def _chunk_widths(extent):
    ws = []
    e = extent
    while e >= 512:
        ws.append(512)
        e -= 512
    if e:
        ws.append(e)          # 128/256/384 tail (multiple of 128)
    return ws


def _build(extents):
    nc = bacc.Bacc("TRN2", target_bir_lowering=False, debug=False,
                   num_devices=N_CORES)
    SX = sum(extents)
    offs = np.cumsum([0] + list(extents))[:-1]

    qT = nc.dram_tensor("qT", [D, SX], F16, kind="ExternalInput")
    kT = nc.dram_tensor("kT", [D, NSLOT * KK], F16, kind="ExternalInput")
    vA = nc.dram_tensor("vA", [NSLOT, KK, D + 1], F16, kind="ExternalInput")
    out = nc.dram_tensor("out", [SX, D + 1], F16, kind="ExternalOutput")

    active = [i for i in range(NSLOT) if extents[i] > 0]

    with tile.TileContext(nc) as tc, ExitStack() as ctx:
        kv_pool = ctx.enter_context(tc.tile_pool(name="kv", bufs=1))
        q_pool = ctx.enter_context(tc.tile_pool(name="q", bufs=1))
        e_pool = ctx.enter_context(tc.tile_pool(name="e", bufs=1))
        o_pool = ctx.enter_context(tc.tile_pool(name="o", bufs=6))
        s_psum = ctx.enter_context(tc.tile_pool(name="sps", bufs=3, space="PSUM"))
        o_psum = ctx.enter_context(tc.tile_pool(name="ops", bufs=2, space="PSUM"))

        kt_tiles = {}
        vt_tiles = {}
        qs_tiles = {}
        bal = {"ACT": 0.0, "DVE": 0.0}
        e_ctr = [0]

        def load_slot(i, eng):
            kt = kv_pool.tile([D, KK], F16, name=f"kt{i}", tag=f"kt{i}")
            eng.dma_start(out=kt, in_=kT[:, i * KK:(i + 1) * KK])
            vt = kv_pool.tile([128, NKT, D + 1], F16, name=f"vt{i}", tag=f"vt{i}")
            nc.gpsimd.dma_start(
                out=vt,
                in_=vA[i].rearrange("(t p) d -> p t d", p=128))
            kt_tiles[i] = kt
            vt_tiles[i] = vt

        def load_q(i, eng, split=False):
            e = extents[i]
            qs = q_pool.tile([D, e], F16, name=f"qs{i}", tag=f"qs{i}")
            if split and e > 512:
                eng.dma_start(out=qs[:, 0:512],
                              in_=qT[:, offs[i]:offs[i] + 512])
                eng.dma_start(out=qs[:, 512:e],
                              in_=qT[:, offs[i] + 512:offs[i] + e])
            else:
                eng.dma_start(out=qs, in_=qT[:, offs[i]:offs[i] + e])
            qs_tiles[i] = qs

        def pick(cost_act, cost_dve):
            if bal["ACT"] + cost_act <= bal["DVE"] + cost_dve:
                bal["ACT"] += cost_act
                return "ACT"
            bal["DVE"] += cost_dve
            return "DVE"

        def scores_exp_groups(slot, q0, w):
            """Return closures, each emitting one ktile-pair's score matmuls
            + its exp, and the list of e-tiles filled."""
            qt = qs_tiles[slot][:, q0:q0 + w]
            kt = kt_tiles[slot]
            ets = []

            def make_group(jt):
                e_t = e_pool.tile([128, 2, w], F16, name=f"et{e_ctr[0] % 10}",
                                  tag=f"et{e_ctr[0] % 10}",
                                  padded_shape=[128, 2, 512])
                e_ctr[0] += 1
                ets.append(e_t)

                def g():
                  with tc.high_priority(offset=300):
                    s_ps = s_psum.tile([128, 2, w], F32, name="s_ps", tag="s_ps",
                                       padded_shape=[128, 2, 512])
                    for u in range(2):
                        j = 2 * jt + u
                        nc.tensor.matmul(
                            s_ps[:, u, :],
                            kt[:, j * 128:(j + 1) * 128],
                            qt,
                            start=True, stop=True)
                    eng = pick(_exp_cost_act(2 * w), _exp_cost_dve(2 * w))
                    if eng == "ACT":
                        nc.scalar.activation(
                            e_t, s_ps, mybir.ActivationFunctionType.Exp,
                            bias=0.0, scale=1.0)
                    else:
                        nc.vector.tensor_scalar(
                            e_t.bitcast(I16), s_ps, A16, SCH_BIAS,
                            mybir.AluOpType.mult, mybir.AluOpType.add)
                return g

            return [make_group(jt) for jt in range(2)], ets

        def pv_parts(slot, q0, w, ets):
            """Per-o_ps-group closures (8 PV matmuls + PSUM exit copy into a
            chunk-wide SBUF tile) and a finalizer emitting one store."""
            vt = vt_tiles[slot]
            nst = w // 128
            ogs = [(t0, min(2, nst - t0)) for t0 in range(0, nst, 2)]
            o_sb = o_pool.tile([128, nst, D + 1], F16, name="o_sb", tag="o_sb",
                               padded_shape=[128, 4, D + 1])

            def make_og(t0, tn):
                def g():
                    o_ps = o_psum.tile([128, tn, D + 1], F32, name="o_ps",
                                       padded_shape=[128, 2, D + 1])
                    for t2 in range(tn):
                        t = t0 + t2
                        for j in range(NKT):
                            e_t = ets[j // 2]
                            nc.tensor.matmul(
                                o_ps[:, t2, :],
                                e_t[:, j % 2, t * 128:(t + 1) * 128],
                                vt[:, j, :],
                                start=(t2 == 0 and j == 0),
                                stop=(t2 == tn - 1 and j == NKT - 1))
                    eng = pick(_cp_cost_act(tn * (D + 1)), _cp_cost_dve(tn * (D + 1)))
                    if eng == "ACT":
                        nc.scalar.activation(
                            o_sb[:, t0:t0 + tn, :], o_ps,
                            mybir.ActivationFunctionType.Copy,
                            bias=0.0, scale=1.0)
                    else:
                        nc.vector.tensor_scalar_mul(o_sb[:, t0:t0 + tn, :],
                                                    o_ps, 1.0)
                return g

            def fin(eng=None):
                r0 = offs[slot] + q0
                (eng or nc.gpsimd).dma_start(
                    out=out[r0:r0 + nst * 128, :].rearrange(
                        "(t p) d -> p t d", p=128),
                    in_=o_sb)

            return [make_og(t0, tn) for t0, tn in ogs], fin

        chunks = []
        for i in active:
            q0 = 0
            for w in _chunk_widths(extents[i]):
                chunks.append((i, q0, w))
                q0 += w

        # Queue all K/Q/V loads up front, interleaved by slot parity across
        # the SP and ACT HWDGE queues (each queue serves ~64GB/s, so a
        # single queue cannot keep up with the PE's ~55GB/s Q consumption
        # plus K; two queues prefetch slots well ahead of use). V rides the
        # gpsimd SWDGE queue. Slot 0's Q is split so its first chunk lands
        # fast.
        for n, i in enumerate(active):
            eng = nc.sync if n % 2 == 0 else nc.scalar
            load_slot(i, eng)
            load_q(i, eng, split=(n == 0))

        pending_fins = []
        prev = None
        for ci, (slot, q0, w) in enumerate(chunks):
            groups, ets = scores_exp_groups(slot, q0, w)
            subs, fin = pv_parts(*prev) if prev is not None else ([], None)
            G, T = len(groups), len(subs)
            a = bi = 0
            while a < G or bi < T:
                if a < G and (T == 0 or a * T <= (bi + 1) * G):
                    groups[a]()
                    a += 1
                else:
                    subs[bi]()
                    bi += 1
            if fin is not None:
                pending_fins.append(fin)
            if len(pending_fins) > 2:
                pending_fins.pop(0)()
            prev = (slot, q0, w, ets)
        if prev is not None:
            subs, fin = pv_parts(*prev)
            for s_ in subs:
                s_()
            pending_fins.append(fin)
        for f in pending_fins:
            f(nc.sync)

    nc.compile()
    return nc


def _get_compiled(extents):
    key = tuple(extents)
    if key not in _COMPILED:
        _COMPILED[key] = _build(key)
    return _COMPILED[key]


def _plan(valid_len):
    """Sort batches by valid_len desc; slot i pairs order[2i] (group 0) with
    order[2i+1] (group 1); extent[i] = rounded pair max."""
    vl = np.asarray(valid_len).astype(np.int64)
    order = np.argsort(-vl, kind="stable")
    extents = []
    for i in range(NSLOT):
        m = int(vl[order[2 * i]])     # sorted desc -> pair max is first
        extents.append(min(S, -(-m // 128) * 128))
    return order, extents


def run_sharded(queries, keys, values, valid_len, **spmd_kwargs):
    """Run the kernel on 8 cores; returns (full_output, BassKernelResults)."""
    q = np.asarray(queries, dtype=np.float32)
    k = np.asarray(keys, dtype=np.float32)
    v = np.asarray(values, dtype=np.float32)
    vl = np.asarray(valid_len).astype(np.int64)

    order, extents = _plan(vl)
    if not any(extents):
        return np.broadcast_to(v.mean(axis=1)[:, None, :],
                               (B, S, D)).astype(np.float32).copy(), None
    nc = _get_compiled(extents)
    SX = sum(extents)
    offs = np.cumsum([0] + list(extents))[:-1]

    mask = (np.arange(S)[None, :] < vl[:, None]).astype(np.float32)
    scale = np.float32(1.0 / np.sqrt(D))
    qm = (q * (mask * scale)[:, :, None]).astype(np.float16)   # [B, S, D]
    kh = k.astype(np.float16)
    vA = np.concatenate([v, np.ones((B, S, 1), np.float32)],
                        axis=2).astype(np.float16)              # [B, S, D+1]

    in_maps = []
    for g in range(NG):
        bsel = [int(order[2 * i + g]) for i in range(NSLOT)]
        qT = np.empty((D, SX), np.float16)
        for i in range(NSLOT):
            e = extents[i]
            if e:
                qT[:, offs[i]:offs[i] + e] = qm[bsel[i], :e].T
        for h in range(KQ):
            kTg = np.empty((D, NSLOT * KK), np.float16)
            vAg = np.empty((NSLOT, KK, D + 1), np.float16)
            for i in range(NSLOT):
                kTg[:, i * KK:(i + 1) * KK] = kh[bsel[i], h * KK:(h + 1) * KK].T
                vAg[i] = vA[bsel[i], h * KK:(h + 1) * KK]
            in_maps.append({"qT": np.ascontiguousarray(qT),
                            "kT": np.ascontiguousarray(kTg),
                            "vA": np.ascontiguousarray(vAg)})

    res = run_bass_kernel_spmd(nc, in_maps, list(range(N_CORES)), **spmd_kwargs)

    vmean = v.mean(axis=1)                                      # [B, D]
    full = np.empty((B, S, D), np.float32)
    for g in range(NG):
        tot = np.zeros((SX, D + 1), np.float32)
        for h in range(KQ):
            tot += res.results[g * KQ + h]["out"].astype(np.float32)
        for i in range(NSLOT):
            b = int(order[2 * i + g])
            n = int(min(vl[b], extents[i]))
            if n:
                blk = tot[offs[i]:offs[i] + n]
                full[b, :n] = blk[:, :D] / blk[:, D:D + 1]
            if n < S:
                full[b, n:] = vmean[b]
    return full, res


def kernel(queries, keys, values, valid_len):
    out, _ = run_sharded(queries, keys, values, valid_len)
    return out


# revision 14
# speedup vs baseline: 1.3739x; 1.3739x over previous
"""Trainium2 Bass kernel for batched dot-product attention with query-row
masking (nn_DotProductAttention: B=32, Q=K=2048, D=128, fp32).

Strategy (v2)
-------------
- 2D sharding over 8 cores: 2 batch-groups x 4 K-quarters. Batches are
  sorted by valid_len and paired into 16 slots (one batch per group per
  slot); each core handles its group's 16 batches against one 512-key
  quarter. Per-core query extent = sum of pairwise slot maxes (~16.8k rows)
  -- ~20% less PE work than 8-way batch-parallel slot maxes, and the
  outputs are partial (num, den) sums the host combines, so the split
  needs no collectives.
- The reference masks whole QUERY rows; mask and 1/sqrt(D) are folded into
  Q on the host (masked rows -> zero queries -> uniform softmax), and rows
  beyond each slot's extent are filled on the host with mean(V).
- Device per chunk (512 q rows): fp16 score matmuls per 128-key tile into
  [128, 2, 512] PSUM tiles; exp is split between ScalarE (native Exp,
  bias -1) and DVE (Schraudolph bit-trick: i16 = trunc(1477.32*s +
  13883.18) bitcast to fp16 ~= exp(s-1)), chosen per tile by a running
  busy-time balance -- this halves the ScalarE exp wall that bounded v1.
- PV: fp16 matmuls with the exp tiles as weights against [V | 1] (the ones
  column accumulates the softmax denominator); one PSUM accumulation group
  spans two 128-row subtiles of one bank ([128, 2, 129]). PSUM->SBUF exit
  copies (fp16) are also balance-assigned to ScalarE/DVE. Normalization
  (num/den) happens on the host, which also sums the 4 K-quarter partials.
- DMA: K/V for all 16 slots are SBUF-resident (loaded once at start); Q
  chunks stream on the SP HWDGE queue; V loads and output stores ride the
  gpsimd SWDGE queues with stores emitted two chunks late (in-order queue
  must not park on unfinished PV results); tail flush on SP.
"""

import sys

for _p in ("/opt/trn_rl_repo", "/root/.axon_site/_ro/trn_rl_repo"):
    if _p not in sys.path:
        sys.path.append(_p)

import math
from contextlib import ExitStack

import numpy as np

import concourse.bacc as bacc
import concourse.tile as tile
from concourse import mybir
from concourse.bass_utils import run_bass_kernel_spmd

B, S, D = 32, 2048, 128
N_CORES = 8
NG = 2                    # batch groups
NSLOT = B // NG           # 16 slots (one batch per group each)
KQ = N_CORES // NG        # 4 K-quarters
KK = S // KQ              # 512 keys per core
NKT = KK // 128           # 4 key tiles per core
F32 = mybir.dt.float32
F16 = mybir.dt.float16
I16 = mybir.dt.int16

A16 = 1024.0 / math.log(2.0)          # 1477.3196
SCH_BIAS = 15360.0 + 0.5              # schraudolph exp(s), trunc-centered

_COMPILED = {}

# rough per-instruction busy-ns models used for ACT/DVE load balancing
def _exp_cost_act(cols):
    return cols * 0.8333 + 420.0

def _exp_cost_dve(cols):
    return cols * 1.0417 + 300.0

def _cp_cost_act(cols):
    return cols * 0.8333 + 380.0

def _cp_cost_dve(cols):
    return cols * 1.0417 + 250.0


def _chunk_widths(extent):
    ws = []
    e = extent
    while e >= 512:
        ws.append(512)
        e -= 512
    if e:
        ws.append(e)          # 128/256/384 tail (multiple of 128)
    return ws


def _build(extents):
    nc = bacc.Bacc("TRN2", target_bir_lowering=False, debug=False,
                   num_devices=N_CORES)
    SX = sum(extents)
    offs = np.cumsum([0] + list(extents))[:-1]

    qT = nc.dram_tensor("qT", [D, SX], F16, kind="ExternalInput")
    kT = nc.dram_tensor("kT", [D, NSLOT * KK], F16, kind="ExternalInput")
    vA = nc.dram_tensor("vA", [NSLOT, KK, D + 1], F16, kind="ExternalInput")
    out = nc.dram_tensor("out", [SX, D + 1], F16, kind="ExternalOutput")

    active = [i for i in range(NSLOT) if extents[i] > 0]

    with tile.TileContext(nc) as tc, ExitStack() as ctx:
        kv_pool = ctx.enter_context(tc.tile_pool(name="kv", bufs=1))
        q_pool = ctx.enter_context(tc.tile_pool(name="q", bufs=1))
        e_pool = ctx.enter_context(tc.tile_pool(name="e", bufs=1))
        o_pool = ctx.enter_context(tc.tile_pool(name="o", bufs=6))
        s_psum = ctx.enter_context(tc.tile_pool(name="sps", bufs=3, space="PSUM"))
        o_psum = ctx.enter_context(tc.tile_pool(name="ops", bufs=2, space="PSUM"))

        kt_tiles = {}
        vt_tiles = {}
        qs_tiles = {}
        bal = {"ACT": 0.0, "DVE": 0.0}
        e_ctr = [0]

        def load_slot(i, eng):
            kt = kv_pool.tile([D, KK], F16, name=f"kt{i}", tag=f"kt{i}")
            eng.dma_start(out=kt, in_=kT[:, i * KK:(i + 1) * KK])
            vt = kv_pool.tile([128, NKT, D + 1], F16, name=f"vt{i}", tag=f"vt{i}")
            nc.gpsimd.dma_start(
                out=vt,
                in_=vA[i].rearrange("(t p) d -> p t d", p=128))
            kt_tiles[i] = kt
            vt_tiles[i] = vt

        def load_q(i, eng, split=False):
            e = extents[i]
            qs = q_pool.tile([D, e], F16, name=f"qs{i}", tag=f"qs{i}")
            if split and e > 512:
                eng.dma_start(out=qs[:, 0:512],
                              in_=qT[:, offs[i]:offs[i] + 512])
                eng.dma_start(out=qs[:, 512:e],
                              in_=qT[:, offs[i] + 512:offs[i] + e])
            else:
                eng.dma_start(out=qs, in_=qT[:, offs[i]:offs[i] + e])
            qs_tiles[i] = qs

        def pick(cost_act, cost_dve):
            if bal["ACT"] + cost_act <= bal["DVE"] + cost_dve:
                bal["ACT"] += cost_act
                return "ACT"
            bal["DVE"] += cost_dve
            return "DVE"

        def scores_exp_groups(slot, q0, w):
            """Return closures, each emitting one ktile-pair's score matmuls
            + its exp, and the list of e-tiles filled."""
            qt = qs_tiles[slot][:, q0:q0 + w]
            kt = kt_tiles[slot]
            ets = []

            def make_group(jt):
                e_t = e_pool.tile([128, 2, w], F16, name=f"et{e_ctr[0] % 10}",
                                  tag=f"et{e_ctr[0] % 10}",
                                  padded_shape=[128, 2, 512])
                e_ctr[0] += 1
                ets.append(e_t)

                def g():
                  with tc.high_priority(offset=300):
                    s_ps = s_psum.tile([128, 2, w], F32, name="s_ps", tag="s_ps",
                                       padded_shape=[128, 2, 512])
                    for u in range(2):
                        j = 2 * jt + u
                        nc.tensor.matmul(
                            s_ps[:, u, :],
                            kt[:, j * 128:(j + 1) * 128],
                            qt,
                            start=True, stop=True)
                    eng = pick(_exp_cost_act(2 * w), _exp_cost_dve(2 * w))
                    if eng == "ACT":
                        nc.scalar.activation(
                            e_t, s_ps, mybir.ActivationFunctionType.Exp,
                            bias=0.0, scale=1.0)
                    else:
                        nc.vector.tensor_scalar(
                            e_t.bitcast(I16), s_ps, A16, SCH_BIAS,
                            mybir.AluOpType.mult, mybir.AluOpType.add)
                return g

            return [make_group(jt) for jt in range(2)], ets

        def pv_parts(slot, q0, w, ets):
            """Per-o_ps-group closures (8 PV matmuls + PSUM exit copy into a
            chunk-wide SBUF tile) and a finalizer emitting one store."""
            vt = vt_tiles[slot]
            nst = w // 128
            ogs = [(t0, min(2, nst - t0)) for t0 in range(0, nst, 2)]
            o_sb = o_pool.tile([128, nst, D + 1], F16, name="o_sb", tag="o_sb",
                               padded_shape=[128, 4, D + 1])

            def make_og(t0, tn):
                def g():
                    o_ps = o_psum.tile([128, tn, D + 1], F32, name="o_ps",
                                       padded_shape=[128, 2, D + 1])
                    for t2 in range(tn):
                        t = t0 + t2
                        for j in range(NKT):
                            e_t = ets[j // 2]
                            nc.tensor.matmul(
                                o_ps[:, t2, :],
                                e_t[:, j % 2, t * 128:(t + 1) * 128],
                                vt[:, j, :],
                                start=(t2 == 0 and j == 0),
                                stop=(t2 == tn - 1 and j == NKT - 1))
                    eng = pick(_cp_cost_act(tn * (D + 1)), _cp_cost_dve(tn * (D + 1)))
                    if eng == "ACT":
                        nc.scalar.activation(
                            o_sb[:, t0:t0 + tn, :], o_ps,
                            mybir.ActivationFunctionType.Copy,
                            bias=0.0, scale=1.0)
                    else:
                        nc.vector.tensor_scalar_mul(o_sb[:, t0:t0 + tn, :],
                                                    o_ps, 1.0)
                return g

            def fin(eng=None):
                r0 = offs[slot] + q0
                (eng or nc.gpsimd).dma_start(
                    out=out[r0:r0 + nst * 128, :].rearrange(
                        "(t p) d -> p t d", p=128),
                    in_=o_sb)

            return [make_og(t0, tn) for t0, tn in ogs], fin

        chunks = []
        for i in active:
            q0 = 0
            for w in _chunk_widths(extents[i]):
                chunks.append((i, q0, w))
                q0 += w

        # Queue all K/Q loads on the SP HWDGE queue in slot order (hardware
        # descriptor generation; the queue streams ~300GB/s on these 1KB-row
        # patterns and prefetches slots well ahead of use). V loads go first
        # on the gpsimd SWDGE queue, ahead of all output stores. Slot 0's Q
        # is split so its first chunk lands fast.
        for n, i in enumerate(active):
            load_slot(i, nc.sync)
            load_q(i, nc.sync, split=(n == 0))

        pending_fins = []
        prev = None
        for ci, (slot, q0, w) in enumerate(chunks):
            groups, ets = scores_exp_groups(slot, q0, w)
            subs, fin = pv_parts(*prev) if prev is not None else ([], None)
            G, T = len(groups), len(subs)
            a = bi = 0
            while a < G or bi < T:
                if a < G and (T == 0 or a * T <= (bi + 1) * G):
                    groups[a]()
                    a += 1
                else:
                    subs[bi]()
                    bi += 1
            if fin is not None:
                pending_fins.append(fin)
            if len(pending_fins) > 2:
                pending_fins.pop(0)()
            prev = (slot, q0, w, ets)
        if prev is not None:
            subs, fin = pv_parts(*prev)
            for s_ in subs:
                s_()
            pending_fins.append(fin)
        for f in pending_fins:
            f(nc.sync)

    nc.compile()
    return nc


def _get_compiled(extents):
    key = tuple(extents)
    if key not in _COMPILED:
        _COMPILED[key] = _build(key)
    return _COMPILED[key]


def _plan(valid_len):
    """Sort batches by valid_len desc; slot i pairs order[2i] (group 0) with
    order[2i+1] (group 1); extent[i] = rounded pair max."""
    vl = np.asarray(valid_len).astype(np.int64)
    order = np.argsort(-vl, kind="stable")
    extents = []
    for i in range(NSLOT):
        m = int(vl[order[2 * i]])     # sorted desc -> pair max is first
        extents.append(min(S, -(-m // 128) * 128))
    return order, extents


def run_sharded(queries, keys, values, valid_len, **spmd_kwargs):
    """Run the kernel on 8 cores; returns (full_output, BassKernelResults)."""
    q = np.asarray(queries, dtype=np.float32)
    k = np.asarray(keys, dtype=np.float32)
    v = np.asarray(values, dtype=np.float32)
    vl = np.asarray(valid_len).astype(np.int64)

    order, extents = _plan(vl)
    if not any(extents):
        return np.broadcast_to(v.mean(axis=1)[:, None, :],
                               (B, S, D)).astype(np.float32).copy(), None
    nc = _get_compiled(extents)
    SX = sum(extents)
    offs = np.cumsum([0] + list(extents))[:-1]

    mask = (np.arange(S)[None, :] < vl[:, None]).astype(np.float32)
    scale = np.float32(1.0 / np.sqrt(D))
    qm = (q * (mask * scale)[:, :, None]).astype(np.float16)   # [B, S, D]
    kh = k.astype(np.float16)
    vA = np.concatenate([v, np.ones((B, S, 1), np.float32)],
                        axis=2).astype(np.float16)              # [B, S, D+1]

    in_maps = []
    for g in range(NG):
        bsel = [int(order[2 * i + g]) for i in range(NSLOT)]
        qT = np.empty((D, SX), np.float16)
        for i in range(NSLOT):
            e = extents[i]
            if e:
                qT[:, offs[i]:offs[i] + e] = qm[bsel[i], :e].T
        for h in range(KQ):
            kTg = np.empty((D, NSLOT * KK), np.float16)
            vAg = np.empty((NSLOT, KK, D + 1), np.float16)
            for i in range(NSLOT):
                kTg[:, i * KK:(i + 1) * KK] = kh[bsel[i], h * KK:(h + 1) * KK].T
                vAg[i] = vA[bsel[i], h * KK:(h + 1) * KK]
            in_maps.append({"qT": np.ascontiguousarray(qT),
                            "kT": np.ascontiguousarray(kTg),
                            "vA": np.ascontiguousarray(vAg)})

    res = run_bass_kernel_spmd(nc, in_maps, list(range(N_CORES)), **spmd_kwargs)

    vmean = v.mean(axis=1)                                      # [B, D]
    full = np.empty((B, S, D), np.float32)
    for g in range(NG):
        tot = np.zeros((SX, D + 1), np.float32)
        for h in range(KQ):
            tot += res.results[g * KQ + h]["out"].astype(np.float32)
        for i in range(NSLOT):
            b = int(order[2 * i + g])
            n = int(min(vl[b], extents[i]))
            if n:
                blk = tot[offs[i]:offs[i] + n]
                full[b, :n] = blk[:, :D] / blk[:, D:D + 1]
            if n < S:
                full[b, n:] = vmean[b]
    return full, res


def kernel(queries, keys, values, valid_len):
    out, _ = run_sharded(queries, keys, values, valid_len)
    return out
